# revision 6
# baseline (speedup 1.0000x reference)
"""LSTM cell kernel for Trainium2, SPMD over 8 NeuronCores.

Problem: nn_LstmCell — B=8192, D_IN=D_H=2048.
    g = x @ Wx.T + bx + h @ Wh.T + bh          # [B, 3H]
    gi, gm, go = split(g, 3)
    c_new = sigmoid(gm)*c + sigmoid(gi)*tanh(gm)
    h_new = sigmoid(go)*tanh(c_new)

Strategy:
  - Data-parallel over batch: each core owns 1024 rows of x/h/c.
  - Single fused GEMM: A = [x ‖ h] (K=4096), W = [Wx ‖ Wh] ([6144, 4096]).
    Computed transposed (gates on PSUM partitions, batch on free dim) so the
    per-gate bias folds into the ScalarE activation (per-partition bias) and
    sigmoid/tanh read PSUM directly.
  - bf16 matmul inputs (fp32 PSUM accumulation); elementwise math in fp32.
  - Weights streamed from HBM (one pass), activations resident in SBUF.

Host-side: layout transforms + bf16 casts (not counted in HW exec time).
"""

import os

import numpy as np
import ml_dtypes

N_CORES = 8
B = 8192
DH = 2048            # latent dim (= D_IN = D_H)
H3 = 3 * DH          # 6144 gate rows
K = 2 * DH           # 4096 contraction dim
BLOC = B // N_CORES  # 1024 batch rows per core
P = 128
KT = K // P          # 32 k-tiles
MT = H3 // P         # 48 gate-row tiles
DTL = DH // P        # 16 d-tiles per gate
NF = 512             # matmul free dim (one PSUM bank of fp32)
NH = BLOC // NF      # 2 batch halves

_BF16 = ml_dtypes.bfloat16

_CACHE = {}
LAST_RESULT = None  # BassKernelResults from the most recent run (for test.py)


def _split_multiwaits(nc):
    """This container's walrus build rejects >1 sync-wait on an engine
    instruction ("Too many sync wait commands"). Split extra waits into
    standalone EventSemaphore instructions on the same engine immediately
    before the instruction (same stall semantics: engines are in-order)."""
    import concourse.mybir as mybir

    f = nc.m.functions[0]
    for blk in f.blocks:
        new_insts = []
        for inst in blk.instructions:
            si = getattr(inst, "sync_info", None)
            ow = list(si.on_wait) if (si is not None and si.on_wait) else []
            if len(ow) > 1:
                for w in ow[:-1]:
                    new_insts.append(
                        mybir.InstEventSemaphore(
                            name=nc.get_next_instruction_name(),
                            engine=inst.engine,
                            ins=[],
                            outs=[],
                            sync_info=mybir.SyncInfo(on_wait=[w], on_update=[]),
                        )
                    )
                inst.sync_info = mybir.SyncInfo(
                    on_wait=[ow[-1]], on_update=list(si.on_update)
                )
            new_insts.append(inst)
        blk.instructions[:] = new_insts


def _build_bass():
    import concourse.bass as bass
    import concourse.mybir as mybir
    import concourse.tile as tile

    f32 = mybir.dt.float32
    bf16 = mybir.dt.bfloat16
    AF = mybir.ActivationFunctionType

    nc = bass.Bass("TRN2", name="lstm_cell")

    WH = nc.dram_tensor("WH", [MT, P, KT, P], bf16, kind="ExternalInput")
    AH = nc.dram_tensor("AH", [P, KT, BLOC], bf16, kind="ExternalInput")
    CT = nc.dram_tensor("CT", [DH, BLOC], f32, kind="ExternalInput")
    BIAS = nc.dram_tensor("BIAS", [P, MT], f32, kind="ExternalInput")
    HT = nc.dram_tensor("HT", [DH, BLOC], f32, kind="ExternalOutput")
    CNT = nc.dram_tensor("CNT", [DH, BLOC], f32, kind="ExternalOutput")

    with tile.TileContext(nc) as tc:
        with (
            tc.tile_pool(name="const", bufs=1) as const_pool,
            tc.tile_pool(name="wpool", bufs=2) as wpool,
            tc.tile_pool(name="cpool", bufs=2) as cpool,
            tc.tile_pool(name="epool", bufs=3) as epool,
            tc.tile_pool(name="psum", bufs=1, space="PSUM") as psum_pool,
        ):
            # Activations resident in SBUF; split the load so early k-tiles
            # are available before the whole 8 MB lands.
            a_sb = const_pool.tile([P, KT, BLOC], bf16, name="a_sb")
            for kg in range(4):
                nc.sync.dma_start(
                    a_sb[:, kg * 8 : (kg + 1) * 8, :],
                    AH[:, kg * 8 : (kg + 1) * 8, :],
                )
            bias_sb = const_pool.tile([P, MT], f32, name="bias_sb")
            nc.sync.dma_start(bias_sb[:], BIAS[:])

            for d in range(DTL):
                # Stream this d-tile's three gate weight strips (1 MB each).
                strips = []
                for gi, g in enumerate("imo"):
                    mt = gi * DTL + d
                    w_sb = wpool.tile([P, KT, P], bf16, name=f"w_{g}", tag=f"w_{g}")
                    nc.sync.dma_start(w_sb[:], WH[mt])
                    strips.append(w_sb)

                c_tiles = []
                for nh in range(NH):
                    c_t = cpool.tile([P, NF], f32, name=f"c_{nh}", tag=f"c_{nh}")
                    nc.sync.dma_start(
                        c_t[:], CT[d * P : (d + 1) * P, nh * NF : (nh + 1) * NF]
                    )
                    c_tiles.append(c_t)

                # GEMM: 3 gates x 32 k-tiles x 2 batch halves.
                # One weight load feeds both batch halves.
                psums = {}
                for gi, g in enumerate("imo"):
                    w_sb = strips[gi]
                    for nh in range(NH):
                        psums[(g, nh)] = psum_pool.tile(
                            [P, NF], f32, name=f"ps_{g}{nh}", tag=f"ps_{g}{nh}"
                        )
                    for kt in range(KT):
                        for nh in range(NH):
                            nc.tensor.matmul(
                                psums[(g, nh)][:],
                                w_sb[:, kt, :],
                                a_sb[:, kt, nh * NF : (nh + 1) * NF],
                                start=(kt == 0),
                                stop=(kt == KT - 1),
                            )

                # Epilogue: gates + cell update, fp32.
                for nh in range(NH):
                    b_i = bias_sb[:, d : d + 1]
                    b_m = bias_sb[:, DTL + d : DTL + d + 1]
                    b_o = bias_sb[:, 2 * DTL + d : 2 * DTL + d + 1]

                    s_i = epool.tile([P, NF], f32, name="s_i", tag="s_i")
                    t_m = epool.tile([P, NF], f32, name="t_m", tag="t_m")
                    s_m = epool.tile([P, NF], f32, name="s_m", tag="s_m")
                    s_o = epool.tile([P, NF], f32, name="s_o", tag="s_o")
                    part = epool.tile([P, NF], f32, name="part", tag="part")
                    fc = epool.tile([P, NF], f32, name="fc", tag="fc")
                    c_new = epool.tile([P, NF], f32, name="c_new", tag="c_new")
                    t_c = epool.tile([P, NF], f32, name="t_c", tag="t_c")
                    h_new = epool.tile([P, NF], f32, name="h_new", tag="h_new")

                    nc.scalar.activation(s_i[:], psums[("i", nh)][:], AF.Sigmoid, bias=b_i)
                    nc.scalar.activation(t_m[:], psums[("m", nh)][:], AF.Tanh, bias=b_m)
                    nc.scalar.activation(s_m[:], psums[("m", nh)][:], AF.Sigmoid, bias=b_m)
                    nc.scalar.activation(s_o[:], psums[("o", nh)][:], AF.Sigmoid, bias=b_o)
                    nc.vector.tensor_mul(part[:], s_i[:], t_m[:])
                    nc.vector.tensor_mul(fc[:], s_m[:], c_tiles[nh][:])
                    nc.vector.tensor_add(c_new[:], fc[:], part[:])
                    nc.scalar.activation(t_c[:], c_new[:], AF.Tanh)
                    nc.vector.tensor_mul(h_new[:], s_o[:], t_c[:])

                    nc.sync.dma_start(
                        CNT[d * P : (d + 1) * P, nh * NF : (nh + 1) * NF], c_new[:]
                    )
                    nc.sync.dma_start(
                        HT[d * P : (d + 1) * P, nh * NF : (nh + 1) * NF], h_new[:]
                    )

    _split_multiwaits(nc)
    return nc


def _get_bass():
    if "nc" not in _CACHE:
        _CACHE["nc"] = _build_bass()
    return _CACHE["nc"]


def _prepare_in_maps(x, h, c, Wix, bix, Wmx, bmx, Wox, box, Wih, bih, Wmh, bmh, Woh, boh):
    x = np.asarray(x, dtype=np.float32)
    h = np.asarray(h, dtype=np.float32)
    c = np.asarray(c, dtype=np.float32)

    # W = [Wx ‖ Wh] with gate rows [i, m, o]: [6144, 4096]
    W_full = np.concatenate(
        [
            np.concatenate([np.asarray(Wix), np.asarray(Wmx), np.asarray(Wox)], axis=0),
            np.concatenate([np.asarray(Wih), np.asarray(Wmh), np.asarray(Woh)], axis=0),
        ],
        axis=1,
    ).astype(np.float32)
    # WH[mt, p, kt, f] = W_full[mt*128+f, kt*128+p]
    WH_host = np.ascontiguousarray(
        W_full.reshape(MT, P, KT, P).transpose(0, 3, 2, 1)
    ).astype(_BF16)

    # A = [x ‖ h] : [8192, 4096] -> per-core [p, kt, n]
    A = np.concatenate([x, h], axis=1)
    AH_host = np.ascontiguousarray(
        A.reshape(N_CORES, BLOC, KT, P).transpose(0, 3, 2, 1)
    ).astype(_BF16)

    # c transposed per core: [core, 2048, 1024]
    CT_host = np.ascontiguousarray(c.reshape(N_CORES, BLOC, DH).transpose(0, 2, 1))

    bias = np.concatenate(
        [
            np.asarray(bix) + np.asarray(bih),
            np.asarray(bmx) + np.asarray(bmh),
            np.asarray(box) + np.asarray(boh),
        ]
    ).astype(np.float32)
    BIAS_host = np.ascontiguousarray(bias.reshape(MT, P).T)

    return [
        {
            "WH": WH_host,
            "AH": AH_host[core],
            "CT": CT_host[core],
            "BIAS": BIAS_host,
        }
        for core in range(N_CORES)
    ]


def _postprocess(results):
    """results: per-core list of {'HT': [2048,1024], 'CNT': [2048,1024]}."""
    h_new = (
        np.stack([np.asarray(results[core]["HT"]) for core in range(N_CORES)])
        .transpose(0, 2, 1)
        .reshape(B, DH)
        .astype(np.float32)
    )
    c_new = (
        np.stack([np.asarray(results[core]["CNT"]) for core in range(N_CORES)])
        .transpose(0, 2, 1)
        .reshape(B, DH)
        .astype(np.float32)
    )
    return (h_new, c_new)


def kernel(x, h, c, Wix, bix, Wmx, bmx, Wox, box, Wih, bih, Wmh, bmh, Woh, boh):
    global LAST_RESULT
    from concourse.bass_utils import run_bass_kernel_spmd

    in_maps = _prepare_in_maps(
        x, h, c, Wix, bix, Wmx, bmx, Wox, box, Wih, bih, Wmh, bmh, Woh, boh
    )
    nc = _get_bass()
    res = run_bass_kernel_spmd(nc, in_maps, core_ids=list(range(N_CORES)))
    LAST_RESULT = res
    return _postprocess(res.results)


# revision 11
# speedup vs baseline: 10.5485x; 10.5485x over previous
"""LSTM cell kernel for Trainium2, SPMD over 8 NeuronCores.

Problem: nn_LstmCell — B=8192, D_IN=D_H=2048.
    g = x @ Wx.T + bx + h @ Wh.T + bh          # [B, 3H]
    gi, gm, go = split(g, 3)
    c_new = sigmoid(gm)*c + sigmoid(gi)*tanh(gm)
    h_new = sigmoid(go)*tanh(c_new)

Strategy:
  - Data-parallel over batch: each core owns 1024 rows of x/h/c.
  - Single fused GEMM: A = [x ‖ h] (K=4096), W = [Wx ‖ Wh] ([6144, 4096]).
    Computed transposed (gates on PSUM partitions, batch on free dim) so the
    per-gate bias folds into the ScalarE activation (per-partition bias) and
    sigmoid/tanh read PSUM directly.
  - bf16 matmul inputs (fp32 PSUM accumulation); elementwise math in fp32.
  - Weights streamed from HBM (one pass), activations resident in SBUF.

Host-side: layout transforms + bf16 casts (not counted in HW exec time).
"""

import os

import numpy as np
import ml_dtypes

N_CORES = 8
B = 8192
DH = 2048            # latent dim (= D_IN = D_H)
H3 = 3 * DH          # 6144 gate rows
K = 2 * DH           # 4096 contraction dim
BLOC = B // N_CORES  # 1024 batch rows per core
P = 128
KT = K // P          # 32 k-tiles
MT = H3 // P         # 48 gate-row tiles
DTL = DH // P        # 16 d-tiles per gate
NF = 512             # matmul free dim (one PSUM bank of fp32)
NH = BLOC // NF      # 2 batch halves

_BF16 = ml_dtypes.bfloat16

_CACHE = {}
LAST_RESULT = None  # BassKernelResults from the most recent run (for test.py)


def _split_multiwaits(nc):
    """This container's walrus build rejects >1 sync-wait on an engine
    instruction ("Too many sync wait commands"). Split extra waits into
    standalone EventSemaphore instructions on the same engine immediately
    before the instruction (same stall semantics: engines are in-order)."""
    import concourse.mybir as mybir

    f = nc.m.functions[0]
    for blk in f.blocks:
        new_insts = []
        for inst in blk.instructions:
            si = getattr(inst, "sync_info", None)
            ow = list(si.on_wait) if (si is not None and si.on_wait) else []
            if len(ow) > 1:
                for w in ow[:-1]:
                    new_insts.append(
                        mybir.InstEventSemaphore(
                            name=nc.get_next_instruction_name(),
                            engine=inst.engine,
                            ins=[],
                            outs=[],
                            sync_info=mybir.SyncInfo(on_wait=[w], on_update=[]),
                        )
                    )
                inst.sync_info = mybir.SyncInfo(
                    on_wait=[ow[-1]], on_update=list(si.on_update)
                )
            new_insts.append(inst)
        blk.instructions[:] = new_insts


def _build_bass(dtl=DTL):
    import concourse.bass as bass
    import concourse.mybir as mybir
    import concourse.tile as tile

    f32 = mybir.dt.float32
    bf16 = mybir.dt.bfloat16
    AF = mybir.ActivationFunctionType

    nc = bass.Bass("TRN2", name="lstm_cell")

    WH = nc.dram_tensor("WH", [MT, P, KT, P], bf16, kind="ExternalInput")
    AH = nc.dram_tensor("AH", [P, KT, BLOC], bf16, kind="ExternalInput")
    CT = nc.dram_tensor("CT", [DH, BLOC], f32, kind="ExternalInput")
    BIAS = nc.dram_tensor("BIAS", [P, MT], f32, kind="ExternalInput")
    HT = nc.dram_tensor("HT", [DH, BLOC], f32, kind="ExternalOutput")
    CNT = nc.dram_tensor("CNT", [DH, BLOC], f32, kind="ExternalOutput")

    with tile.TileContext(nc) as tc:
        with (
            tc.tile_pool(name="const", bufs=1) as const_pool,
            tc.tile_pool(name="wpool", bufs=2) as wpool,
            tc.tile_pool(name="cpool", bufs=2) as cpool,
            tc.tile_pool(name="epool", bufs=3) as epool,
            tc.tile_pool(name="psum", bufs=1, space="PSUM") as psum_pool,
        ):
            # Activations resident in SBUF; per-k-tile chunks so the first
            # d-tile's matmuls can start as soon as early k-tiles land.
            a_sb = const_pool.tile([P, KT, BLOC], bf16, name="a_sb")
            for kg in range(KT):
                nc.sync.dma_start(
                    a_sb[:, kg : kg + 1, :],
                    AH[:, kg : kg + 1, :],
                )
            bias_sb = const_pool.tile([P, MT], f32, name="bias_sb")
            nc.sync.dma_start(bias_sb[:], BIAS[:])

            for d in range(dtl):
                # Stream this d-tile's three gate weight strips (1 MB each).
                strips = []
                for gi, g in enumerate("imo"):
                    mt = gi * DTL + d
                    w_sb = wpool.tile([P, KT, P], bf16, name=f"w_{g}", tag=f"w_{g}")
                    nc.sync.dma_start(w_sb[:], WH[mt])
                    strips.append(w_sb)

                c_tiles = []
                for nh in range(NH):
                    c_t = cpool.tile([P, NF], f32, name=f"c_{nh}", tag=f"c_{nh}")
                    nc.sync.dma_start(
                        c_t[:], CT[d * P : (d + 1) * P, nh * NF : (nh + 1) * NF]
                    )
                    c_tiles.append(c_t)

                # GEMM: 3 gates x 32 k-tiles x 2 batch halves.
                # One weight load feeds both batch halves.
                psums = {}
                for gi, g in enumerate("imo"):
                    for nh in range(NH):
                        psums[(g, nh)] = psum_pool.tile(
                            [P, NF], f32, name=f"ps_{g}{nh}", tag=f"ps_{g}{nh}"
                        )
                if d == 0:
                    # k-major: PE streams right behind the A-chunk DMAs
                    # instead of stalling on the full A load.
                    for kt in range(KT):
                        for gi, g in enumerate("imo"):
                            for nh in range(NH):
                                nc.tensor.matmul(
                                    psums[(g, nh)][:],
                                    strips[gi][:, kt, :],
                                    a_sb[:, kt, nh * NF : (nh + 1) * NF],
                                    start=(kt == 0),
                                    stop=(kt == KT - 1),
                                )
                else:
                    # gate-major: each gate's PSUM bank drains (ACT) while
                    # the next gate's matmuls run.
                    for gi, g in enumerate("imo"):
                        w_sb = strips[gi]
                        for kt in range(KT):
                            for nh in range(NH):
                                nc.tensor.matmul(
                                    psums[(g, nh)][:],
                                    w_sb[:, kt, :],
                                    a_sb[:, kt, nh * NF : (nh + 1) * NF],
                                    start=(kt == 0),
                                    stop=(kt == KT - 1),
                                )

                # Epilogue: gates + cell update, fp32.
                for nh in range(NH):
                    b_i = bias_sb[:, d : d + 1]
                    b_m = bias_sb[:, DTL + d : DTL + d + 1]
                    b_o = bias_sb[:, 2 * DTL + d : 2 * DTL + d + 1]

                    s_i = epool.tile([P, NF], f32, name="s_i", tag="s_i")
                    t_m = epool.tile([P, NF], f32, name="t_m", tag="t_m")
                    s_m = epool.tile([P, NF], f32, name="s_m", tag="s_m")
                    s_o = epool.tile([P, NF], f32, name="s_o", tag="s_o")
                    part = epool.tile([P, NF], f32, name="part", tag="part")
                    fc = epool.tile([P, NF], f32, name="fc", tag="fc")
                    c_new = epool.tile([P, NF], f32, name="c_new", tag="c_new")
                    t_c = epool.tile([P, NF], f32, name="t_c", tag="t_c")
                    h_new = epool.tile([P, NF], f32, name="h_new", tag="h_new")

                    nc.scalar.activation(s_i[:], psums[("i", nh)][:], AF.Sigmoid, bias=b_i)
                    nc.scalar.activation(t_m[:], psums[("m", nh)][:], AF.Tanh, bias=b_m)
                    nc.scalar.activation(s_m[:], psums[("m", nh)][:], AF.Sigmoid, bias=b_m)
                    nc.scalar.activation(s_o[:], psums[("o", nh)][:], AF.Sigmoid, bias=b_o)
                    nc.vector.tensor_mul(part[:], s_i[:], t_m[:])
                    nc.vector.tensor_mul(fc[:], s_m[:], c_tiles[nh][:])
                    nc.vector.tensor_add(c_new[:], fc[:], part[:])
                    nc.scalar.activation(t_c[:], c_new[:], AF.Tanh)
                    nc.vector.tensor_mul(h_new[:], s_o[:], t_c[:])

                    nc.sync.dma_start(
                        CNT[d * P : (d + 1) * P, nh * NF : (nh + 1) * NF], c_new[:]
                    )
                    nc.sync.dma_start(
                        HT[d * P : (d + 1) * P, nh * NF : (nh + 1) * NF], h_new[:]
                    )

    _split_multiwaits(nc)
    return nc


def _get_bass():
    if "nc" not in _CACHE:
        _CACHE["nc"] = _build_bass()
    return _CACHE["nc"]


def _prepare_in_maps(x, h, c, Wix, bix, Wmx, bmx, Wox, box, Wih, bih, Wmh, bmh, Woh, boh):
    x = np.asarray(x, dtype=np.float32)
    h = np.asarray(h, dtype=np.float32)
    c = np.asarray(c, dtype=np.float32)

    # W = [Wx ‖ Wh] with gate rows [i, m, o]: [6144, 4096]
    W_full = np.concatenate(
        [
            np.concatenate([np.asarray(Wix), np.asarray(Wmx), np.asarray(Wox)], axis=0),
            np.concatenate([np.asarray(Wih), np.asarray(Wmh), np.asarray(Woh)], axis=0),
        ],
        axis=1,
    ).astype(np.float32)
    # WH[mt, p, kt, f] = W_full[mt*128+f, kt*128+p]
    WH_host = np.ascontiguousarray(
        W_full.reshape(MT, P, KT, P).transpose(0, 3, 2, 1)
    ).astype(_BF16)

    # A = [x ‖ h] : [8192, 4096] -> per-core [p, kt, n]
    A = np.concatenate([x, h], axis=1)
    AH_host = np.ascontiguousarray(
        A.reshape(N_CORES, BLOC, KT, P).transpose(0, 3, 2, 1)
    ).astype(_BF16)

    # c transposed per core: [core, 2048, 1024]
    CT_host = np.ascontiguousarray(c.reshape(N_CORES, BLOC, DH).transpose(0, 2, 1))

    bias = np.concatenate(
        [
            np.asarray(bix) + np.asarray(bih),
            np.asarray(bmx) + np.asarray(bmh),
            np.asarray(box) + np.asarray(boh),
        ]
    ).astype(np.float32)
    BIAS_host = np.ascontiguousarray(bias.reshape(MT, P).T)

    return [
        {
            "WH": WH_host,
            "AH": AH_host[core],
            "CT": CT_host[core],
            "BIAS": BIAS_host,
        }
        for core in range(N_CORES)
    ]


def _postprocess(results):
    """results: per-core list of {'HT': [2048,1024], 'CNT': [2048,1024]}."""
    h_new = (
        np.stack([np.asarray(results[core]["HT"]) for core in range(N_CORES)])
        .transpose(0, 2, 1)
        .reshape(B, DH)
        .astype(np.float32)
    )
    c_new = (
        np.stack([np.asarray(results[core]["CNT"]) for core in range(N_CORES)])
        .transpose(0, 2, 1)
        .reshape(B, DH)
        .astype(np.float32)
    )
    return (h_new, c_new)


def kernel(x, h, c, Wix, bix, Wmx, bmx, Wox, box, Wih, bih, Wmh, bmh, Woh, boh):
    global LAST_RESULT
    from concourse.bass_utils import run_bass_kernel_spmd

    in_maps = _prepare_in_maps(
        x, h, c, Wix, bix, Wmx, bmx, Wox, box, Wih, bih, Wmh, bmh, Woh, boh
    )
    nc = _get_bass()
    try:
        res = run_bass_kernel_spmd(nc, in_maps, core_ids=list(range(N_CORES)))
    except ModuleNotFoundError:
        # BASS_TRACE under axon needs antenv.axon_hooks, which some
        # containers lack; fall back to an untraced run.
        os.environ["BASS_NEVER_TRACE"] = "1"
        res = run_bass_kernel_spmd(nc, in_maps, core_ids=list(range(N_CORES)))
    LAST_RESULT = res
    return _postprocess(res.results)


# revision 12
# speedup vs baseline: 10.7429x; 1.0184x over previous
"""LSTM cell kernel for Trainium2, SPMD over 8 NeuronCores.

Problem: nn_LstmCell — B=8192, D_IN=D_H=2048.
    g = x @ Wx.T + bx + h @ Wh.T + bh          # [B, 3H]
    gi, gm, go = split(g, 3)
    c_new = sigmoid(gm)*c + sigmoid(gi)*tanh(gm)
    h_new = sigmoid(go)*tanh(c_new)

Strategy:
  - Data-parallel over batch: each core owns 1024 rows of x/h/c.
  - Single fused GEMM: A = [x ‖ h] (K=4096), W = [Wx ‖ Wh] ([6144, 4096]).
    Computed transposed (gates on PSUM partitions, batch on free dim) so the
    per-gate bias folds into the ScalarE activation (per-partition bias) and
    sigmoid/tanh read PSUM directly.
  - bf16 matmul inputs (fp32 PSUM accumulation); elementwise math in fp32.
  - Weights streamed from HBM (one pass), activations resident in SBUF.

Host-side: layout transforms + bf16 casts (not counted in HW exec time).
"""

import os

import numpy as np
import ml_dtypes

N_CORES = 8
B = 8192
DH = 2048            # latent dim (= D_IN = D_H)
H3 = 3 * DH          # 6144 gate rows
K = 2 * DH           # 4096 contraction dim
BLOC = B // N_CORES  # 1024 batch rows per core
P = 128
KT = K // P          # 32 k-tiles
MT = H3 // P         # 48 gate-row tiles
DTL = DH // P        # 16 d-tiles per gate
NF = 512             # matmul free dim (one PSUM bank of fp32)
NH = BLOC // NF      # 2 batch halves

_BF16 = ml_dtypes.bfloat16

_CACHE = {}
LAST_RESULT = None  # BassKernelResults from the most recent run (for test.py)


def _split_multiwaits(nc):
    """This container's walrus build rejects >1 sync-wait on an engine
    instruction ("Too many sync wait commands"). Split extra waits into
    standalone EventSemaphore instructions on the same engine immediately
    before the instruction (same stall semantics: engines are in-order)."""
    import concourse.mybir as mybir

    f = nc.m.functions[0]
    for blk in f.blocks:
        new_insts = []
        for inst in blk.instructions:
            si = getattr(inst, "sync_info", None)
            ow = list(si.on_wait) if (si is not None and si.on_wait) else []
            if len(ow) > 1:
                for w in ow[:-1]:
                    new_insts.append(
                        mybir.InstEventSemaphore(
                            name=nc.get_next_instruction_name(),
                            engine=inst.engine,
                            ins=[],
                            outs=[],
                            sync_info=mybir.SyncInfo(on_wait=[w], on_update=[]),
                        )
                    )
                inst.sync_info = mybir.SyncInfo(
                    on_wait=[ow[-1]], on_update=list(si.on_update)
                )
            new_insts.append(inst)
        blk.instructions[:] = new_insts


def _build_bass(dtl=DTL):
    import concourse.bass as bass
    import concourse.mybir as mybir
    import concourse.tile as tile

    f32 = mybir.dt.float32
    bf16 = mybir.dt.bfloat16
    AF = mybir.ActivationFunctionType

    nc = bass.Bass("TRN2", name="lstm_cell")

    WH = nc.dram_tensor("WH", [MT, P, KT, P], bf16, kind="ExternalInput")
    AH = nc.dram_tensor("AH", [P, KT, BLOC], bf16, kind="ExternalInput")
    CT = nc.dram_tensor("CT", [DH, BLOC], f32, kind="ExternalInput")
    BIAS = nc.dram_tensor("BIAS", [P, MT], f32, kind="ExternalInput")
    HT = nc.dram_tensor("HT", [DH, BLOC], f32, kind="ExternalOutput")
    CNT = nc.dram_tensor("CNT", [DH, BLOC], f32, kind="ExternalOutput")

    with tile.TileContext(nc) as tc:
        with (
            tc.tile_pool(name="const", bufs=1) as const_pool,
            tc.tile_pool(name="wpool", bufs=2) as wpool,
            tc.tile_pool(name="cpool", bufs=2) as cpool,
            tc.tile_pool(name="epool", bufs=3) as epool,
            tc.tile_pool(name="psum", bufs=1, space="PSUM") as psum_pool,
        ):
            # Activations resident in SBUF; per-k-tile chunks so the first
            # d-tile's matmuls can start as soon as early k-tiles land.
            a_sb = const_pool.tile([P, KT, BLOC], bf16, name="a_sb")
            for kg in range(KT):
                # gpsimd queue: cheap issue, and keeps the A load off the SP
                # queue that streams the weight strips.
                nc.gpsimd.dma_start(
                    a_sb[:, kg : kg + 1, :],
                    AH[:, kg : kg + 1, :],
                )
            bias_sb = const_pool.tile([P, MT], f32, name="bias_sb")
            nc.sync.dma_start(bias_sb[:], BIAS[:])

            for d in range(dtl):
                # Stream this d-tile's three gate weight strips (1 MB each).
                strips = []
                for gi, g in enumerate("imo"):
                    mt = gi * DTL + d
                    w_sb = wpool.tile([P, KT, P], bf16, name=f"w_{g}", tag=f"w_{g}")
                    nc.sync.dma_start(w_sb[:], WH[mt])
                    strips.append(w_sb)

                c_tiles = []
                for nh in range(NH):
                    c_t = cpool.tile([P, NF], f32, name=f"c_{nh}", tag=f"c_{nh}")
                    nc.sync.dma_start(
                        c_t[:], CT[d * P : (d + 1) * P, nh * NF : (nh + 1) * NF]
                    )
                    c_tiles.append(c_t)

                # GEMM: 3 gates x 32 k-tiles x 2 batch halves.
                # One weight load feeds both batch halves.
                psums = {}
                for gi, g in enumerate("imo"):
                    for nh in range(NH):
                        psums[(g, nh)] = psum_pool.tile(
                            [P, NF], f32, name=f"ps_{g}{nh}", tag=f"ps_{g}{nh}"
                        )
                if d == 0:
                    # k-major: PE streams right behind the A-chunk DMAs
                    # instead of stalling on the full A load.
                    for kt in range(KT):
                        for gi, g in enumerate("imo"):
                            for nh in range(NH):
                                nc.tensor.matmul(
                                    psums[(g, nh)][:],
                                    strips[gi][:, kt, :],
                                    a_sb[:, kt, nh * NF : (nh + 1) * NF],
                                    start=(kt == 0),
                                    stop=(kt == KT - 1),
                                )
                else:
                    # gate-major: each gate's PSUM bank drains (ACT) while
                    # the next gate's matmuls run.
                    for gi, g in enumerate("imo"):
                        w_sb = strips[gi]
                        for kt in range(KT):
                            for nh in range(NH):
                                nc.tensor.matmul(
                                    psums[(g, nh)][:],
                                    w_sb[:, kt, :],
                                    a_sb[:, kt, nh * NF : (nh + 1) * NF],
                                    start=(kt == 0),
                                    stop=(kt == KT - 1),
                                )

                # Epilogue: gates + cell update, fp32.
                for nh in range(NH):
                    b_i = bias_sb[:, d : d + 1]
                    b_m = bias_sb[:, DTL + d : DTL + d + 1]
                    b_o = bias_sb[:, 2 * DTL + d : 2 * DTL + d + 1]

                    s_i = epool.tile([P, NF], f32, name="s_i", tag="s_i")
                    t_m = epool.tile([P, NF], f32, name="t_m", tag="t_m")
                    s_m = epool.tile([P, NF], f32, name="s_m", tag="s_m")
                    s_o = epool.tile([P, NF], f32, name="s_o", tag="s_o")
                    part = epool.tile([P, NF], f32, name="part", tag="part")
                    fc = epool.tile([P, NF], f32, name="fc", tag="fc")
                    c_new = epool.tile([P, NF], f32, name="c_new", tag="c_new")
                    t_c = epool.tile([P, NF], f32, name="t_c", tag="t_c")
                    h_new = epool.tile([P, NF], f32, name="h_new", tag="h_new")

                    nc.scalar.activation(s_i[:], psums[("i", nh)][:], AF.Sigmoid, bias=b_i)
                    nc.scalar.activation(t_m[:], psums[("m", nh)][:], AF.Tanh, bias=b_m)
                    nc.scalar.activation(s_m[:], psums[("m", nh)][:], AF.Sigmoid, bias=b_m)
                    nc.scalar.activation(s_o[:], psums[("o", nh)][:], AF.Sigmoid, bias=b_o)
                    nc.vector.tensor_mul(part[:], s_i[:], t_m[:])
                    nc.vector.tensor_mul(fc[:], s_m[:], c_tiles[nh][:])
                    nc.vector.tensor_add(c_new[:], fc[:], part[:])
                    nc.scalar.activation(t_c[:], c_new[:], AF.Tanh)
                    nc.vector.tensor_mul(h_new[:], s_o[:], t_c[:])

                    nc.sync.dma_start(
                        CNT[d * P : (d + 1) * P, nh * NF : (nh + 1) * NF], c_new[:]
                    )
                    nc.sync.dma_start(
                        HT[d * P : (d + 1) * P, nh * NF : (nh + 1) * NF], h_new[:]
                    )

    _split_multiwaits(nc)
    return nc


def _get_bass():
    if "nc" not in _CACHE:
        _CACHE["nc"] = _build_bass()
    return _CACHE["nc"]


def _prepare_in_maps(x, h, c, Wix, bix, Wmx, bmx, Wox, box, Wih, bih, Wmh, bmh, Woh, boh):
    x = np.asarray(x, dtype=np.float32)
    h = np.asarray(h, dtype=np.float32)
    c = np.asarray(c, dtype=np.float32)

    # W = [Wx ‖ Wh] with gate rows [i, m, o]: [6144, 4096]
    W_full = np.concatenate(
        [
            np.concatenate([np.asarray(Wix), np.asarray(Wmx), np.asarray(Wox)], axis=0),
            np.concatenate([np.asarray(Wih), np.asarray(Wmh), np.asarray(Woh)], axis=0),
        ],
        axis=1,
    ).astype(np.float32)
    # WH[mt, p, kt, f] = W_full[mt*128+f, kt*128+p]
    WH_host = np.ascontiguousarray(
        W_full.reshape(MT, P, KT, P).transpose(0, 3, 2, 1)
    ).astype(_BF16)

    # A = [x ‖ h] : [8192, 4096] -> per-core [p, kt, n]
    A = np.concatenate([x, h], axis=1)
    AH_host = np.ascontiguousarray(
        A.reshape(N_CORES, BLOC, KT, P).transpose(0, 3, 2, 1)
    ).astype(_BF16)

    # c transposed per core: [core, 2048, 1024]
    CT_host = np.ascontiguousarray(c.reshape(N_CORES, BLOC, DH).transpose(0, 2, 1))

    bias = np.concatenate(
        [
            np.asarray(bix) + np.asarray(bih),
            np.asarray(bmx) + np.asarray(bmh),
            np.asarray(box) + np.asarray(boh),
        ]
    ).astype(np.float32)
    BIAS_host = np.ascontiguousarray(bias.reshape(MT, P).T)

    return [
        {
            "WH": WH_host,
            "AH": AH_host[core],
            "CT": CT_host[core],
            "BIAS": BIAS_host,
        }
        for core in range(N_CORES)
    ]


def _postprocess(results):
    """results: per-core list of {'HT': [2048,1024], 'CNT': [2048,1024]}."""
    h_new = (
        np.stack([np.asarray(results[core]["HT"]) for core in range(N_CORES)])
        .transpose(0, 2, 1)
        .reshape(B, DH)
        .astype(np.float32)
    )
    c_new = (
        np.stack([np.asarray(results[core]["CNT"]) for core in range(N_CORES)])
        .transpose(0, 2, 1)
        .reshape(B, DH)
        .astype(np.float32)
    )
    return (h_new, c_new)


def kernel(x, h, c, Wix, bix, Wmx, bmx, Wox, box, Wih, bih, Wmh, bmh, Woh, boh):
    global LAST_RESULT
    from concourse.bass_utils import run_bass_kernel_spmd

    in_maps = _prepare_in_maps(
        x, h, c, Wix, bix, Wmx, bmx, Wox, box, Wih, bih, Wmh, bmh, Woh, boh
    )
    nc = _get_bass()
    try:
        res = run_bass_kernel_spmd(nc, in_maps, core_ids=list(range(N_CORES)))
    except ModuleNotFoundError:
        # BASS_TRACE under axon needs antenv.axon_hooks, which some
        # containers lack; fall back to an untraced run.
        os.environ["BASS_NEVER_TRACE"] = "1"
        res = run_bass_kernel_spmd(nc, in_maps, core_ids=list(range(N_CORES)))
    LAST_RESULT = res
    return _postprocess(res.results)
